# revision 7
# baseline (speedup 1.0000x reference)
"""RNN-T JointNetwork kernel for 8 Trainium2 NeuronCores.

Math: out[b,t,u,:] = tanh(concat(fe[b,t], gd[b,u])) @ Wj + bj
with fe = f@We+be, gd = g@Wd+bd.

tanh acts elementwise and the concat feeds one GEMM, so the joint GEMM
factorizes exactly:
    out[b,t,u,:] = A[b,t,:] + C[b,u,:]
    A = tanh(f@We+be) @ Wj[:Dm]          (per-(b,t) row)
    C = tanh(g@Wd+bd) @ Wj[Dm:] + bj     (per-(b,u) row)
leaving the kernel bound by the output write.  The rel-err budget (2e-2)
is huge, so the whole device pipeline runs in fp16 and the output is
written to DRAM as fp16 (half the HBM traffic of fp32); the host upcasts
while unsharding.

Sharding: 8 cores, core c owns (b = c//2, t-half = c%2) -> a [128,64,V]
output chunk per core (contiguous 16.8 MB fp16).

On-core plan:
  - all inputs arrive as exact fp16 SBUF images (host-side shard prep
    does the transpose/swizzle) so every input DMA is one flat block;
    issues are spread over both HWDGE queues (sync + scalar)
  - tfT[m,t] = tanh(We^T@f^T + be), tgT[m,u] likewise (PE + ACT)
  - AC[128, 2V] fp16: for each t-half h, rows 0:64 = A[64h:64h+64],
    rows 64:128 = C (+bj).  Evacuated from PSUM in v-half ladders so the
    main loop starts as early as possible; A-half1 is deferred past the
    first output tiles.
  - main loop: 32 output tiles of [128, 2048] covering 256 DRAM rows
    each; partition p holds DRAM rows 2p/2p+1 of the tile (u-pairs), so
    each output DMA descriptor is 4 KB contiguous (full write rate).
    Per tile: 4 matmuls, stationary selector picks A rows/replicates C
    (K=128, exact 0/1 weights built by gpsimd while inputs stream in).
    PSUM -> fp16 staging: DVE takes cols 0:1024, ACT cols 1024:2048,
    concurrently.  One 512 KB output DMA per tile.
"""

import sys

sys.path.insert(0, "/opt/trn_rl_repo")

import numpy as np

import concourse.bacc as bacc
import concourse.mybir as mybir
import concourse.tile as tile
from concourse.bass_utils import run_bass_kernel_spmd

B, T, U = 4, 256, 64
D = 512  # DE = DD = DM
V = 1024
TC = 128  # t rows per core
NCORES = 8
FP32 = mybir.dt.float32
FP16 = mybir.dt.float16
TANH = mybir.ActivationFunctionType.Tanh

_cache = {}


def _build_nc():
    nc = bacc.Bacc("TRN2", target_bir_lowering=False)

    # all fp16 inputs are pre-swizzled SBUF images [128, free]
    fT_d = nc.dram_tensor("fT_c", [128, 4 * TC], FP16, kind="ExternalInput")
    gT_d = nc.dram_tensor("gT_c", [128, 4 * U], FP16, kind="ExternalInput")
    We_d = nc.dram_tensor("We_i", [128, 4 * D], FP16, kind="ExternalInput")
    Wd_d = nc.dram_tensor("Wd_i", [128, 4 * D], FP16, kind="ExternalInput")
    # Wj images: [row-half][col-half], each [128, 4*512]
    Wj_d = [
        [
            nc.dram_tensor(f"Wj{r}{v}", [128, 4 * 512], FP16, kind="ExternalInput")
            for v in range(2)
        ]
        for r in range(2)
    ]
    be_d = nc.dram_tensor("be_i", [128, 4], FP32, kind="ExternalInput")
    bd_d = nc.dram_tensor("bd_i", [128, 4], FP32, kind="ExternalInput")
    bj_d = nc.dram_tensor("bj", [V], FP16, kind="ExternalInput")
    out_d = nc.dram_tensor("out", [TC * U, V], FP16, kind="ExternalOutput")

    with tile.TileContext(nc) as tc:
        with (
            tc.tile_pool(name="const", bufs=1) as cp,
            tc.tile_pool(name="wts", bufs=1) as wp,
        ):
            # Selector bank [128, 16*2*128]: slice (q, uj) is lhsT for
            # output tile k' (q = k'%16), column block uj.  Output
            # partition m holds DRAM rows 2m (uj=0) / 2m+1 (uj=1) of the
            # 256-row tile: t%64 = 4q + m//32, u = 2*(m%32) + uj.
            #   rows 0:64   (A): 1 iff r == 4q + m//32   (uj-independent)
            #   rows 64:128 (C): 1 iff r-64 == 2*(m%32) + uj
            selAC = cp.tile([128, 16 * 2 * 128], FP16, tag="selAC")
            nc.gpsimd.memset(selAC[:], 0.0)
            slA = selAC[0:64, :]
            nc.gpsimd.affine_select(
                out=slA.rearrange("p (q uj a c) -> p q uj a c", q=16, uj=2, a=4),
                in_=slA.rearrange("p (q uj a c) -> p q uj a c", q=16, uj=2, a=4),
                compare_op=mybir.AluOpType.not_equal,
                fill=1.0,
                base=0,
                pattern=[[-4, 16], [0, 2], [-1, 4], [0, 32]],
                channel_multiplier=1,
            )
            slC = selAC[64:128, :]
            nc.gpsimd.affine_select(
                out=slC.rearrange("p (q uj a c) -> p q uj a c", q=16, uj=2, a=4),
                in_=slC.rearrange("p (q uj a c) -> p q uj a c", q=16, uj=2, a=4),
                compare_op=mybir.AluOpType.not_equal,
                fill=1.0,
                base=0,
                pattern=[[0, 16], [-1, 2], [0, 4], [-2, 32]],
                channel_multiplier=1,
            )

            ones1 = cp.tile([1, 64], FP16, tag="ones1")
            nc.gpsimd.memset(ones1[:], 1.0)

            # ---- persistent operands ----
            fT_sb = wp.tile([128, 4 * TC], FP16, tag="fT")
            gT_sb = wp.tile([128, 4 * U], FP16, tag="gT")
            We_sb = wp.tile([128, 4 * D], FP16, tag="We")
            Wd_sb = wp.tile([128, 4 * D], FP16, tag="Wd")
            Wj_sb = [
                [
                    wp.tile([128, 4 * 512], FP16, tag=f"Wj{r}{v}", name=f"Wj{r}{v}")
                    for v in range(2)
                ]
                for r in range(2)
            ]
            be_sb = wp.tile([128, 4], FP32, tag="be")
            bd_sb = wp.tile([128, 4], FP32, tag="bd")
            bj_sb = wp.tile([1, V], FP16, tag="bj")
            tfT = [wp.tile([128, TC], FP16, tag=f"tfT{c}", name=f"tfT{c}") for c in range(4)]
            tgT = [wp.tile([128, U], FP16, tag=f"tgT{c}", name=f"tgT{c}") for c in range(4)]
            # AC[:, h*V:(h+1)*V]: rows 0:64 = A[64h:64h+64], rows 64:128 = C
            AC = wp.tile([128, 2 * V], FP16, tag="AC")

            # small/early inputs on the scalar HWDGE queue, big weights on
            # sync: the two queues issue in parallel.
            nc.scalar.dma_start(fT_sb[:], fT_d[:])
            nc.scalar.dma_start(be_sb[:], be_d[:])
            nc.scalar.dma_start(gT_sb[:], gT_d[:])
            nc.scalar.dma_start(bd_sb[:], bd_d[:])
            nc.scalar.dma_start(bj_sb[:], bj_d.rearrange("(p v) -> p v", p=1))
            nc.sync.dma_start(We_sb[:], We_d[:])
            nc.sync.dma_start(Wd_sb[:], Wd_d[:])
            for r in range(2):
                for v in range(2):
                    nc.sync.dma_start(Wj_sb[r][v][:], Wj_d[r][v][:])

            # ---- prologue: tfT, tgT, then AC (A-half0 + C), laddered ----
            with tc.tile_pool(name="pp", bufs=4, space="PSUM") as pp:
                for mc in range(4):
                    ps = pp.tile([128, TC], FP32, tag="pps")
                    for dc in range(4):
                        nc.tensor.matmul(
                            ps[:],
                            We_sb[:, dc * D + mc * 128 : dc * D + (mc + 1) * 128],
                            fT_sb[:, dc * TC : (dc + 1) * TC],
                            start=(dc == 0),
                            stop=(dc == 3),
                        )
                    nc.scalar.activation(
                        tfT[mc][:], ps[:], TANH, bias=be_sb[:, mc : mc + 1]
                    )
                for mc in range(4):
                    ps = pp.tile([128, U], FP32, tag="pps")
                    for dc in range(4):
                        nc.tensor.matmul(
                            ps[:],
                            Wd_sb[:, dc * D + mc * 128 : dc * D + (mc + 1) * 128],
                            gT_sb[:, dc * U : (dc + 1) * U],
                            start=(dc == 0),
                            stop=(dc == 3),
                        )
                    nc.scalar.activation(
                        tgT[mc][:], ps[:], TANH, bias=bd_sb[:, mc : mc + 1]
                    )

                # A half 0 -> psum rows 0:64, evacuated per v-half (DVE)
                psA = pp.tile([64, V], FP32, tag="pps")
                for vh in range(2):
                    vs = slice(vh * 512, (vh + 1) * 512)
                    for mc in range(4):
                        nc.tensor.matmul(
                            psA[:, vs],
                            tfT[mc][:, 0:64],
                            Wj_sb[0][vh][:, mc * 512 : (mc + 1) * 512],
                            start=(mc == 0),
                            stop=(mc == 3),
                        )
                    nc.vector.tensor_copy(AC[0:64, vs], psA[:, vs])

                # C -> psum rows 64:128, + bj row; h0 slice per v-half on
                # ACT (laddered), h1 slice in one DVE copy (needed later)
                psC = pp.tile([128, V], FP32, tag="pps")
                for vh in range(2):
                    vs = slice(vh * 512, (vh + 1) * 512)
                    for mc in range(4):
                        nc.tensor.matmul(
                            psC[64:128, vs],
                            tgT[mc][:],
                            Wj_sb[1][vh][:, mc * 512 : (mc + 1) * 512],
                            start=(mc == 0),
                            stop=False,
                            tile_position=(0, 64),
                        )
                    nc.tensor.matmul(
                        psC[64:128, vs],
                        ones1[:],
                        bj_sb[:, vs],
                        start=False,
                        stop=True,
                        tile_position=(0, 64),
                    )
                    nc.scalar.copy(AC[64:128, vs], psC[64:128, vs])
                nc.vector.tensor_copy(AC[64:128, V : 2 * V], psC[64:128, :])

            # ---- main loop: 32 output tiles of [128, 2048] fp16 ----
            # tile k' covers DRAM rows [256k', 256k'+256); partition p
            # holds rows 2p/2p+1 as column halves -> 4 KB descriptors.
            with (
                tc.tile_pool(name="po", bufs=2, space="PSUM") as po,
                tc.tile_pool(name="ob", bufs=4) as ob,
            ):
                for k in range(32):
                    h, q = k // 16, k % 16
                    ps = po.tile([128, 2 * V], FP32, tag="psO")
                    for uj in range(2):
                        lhs = selAC[:, (q * 2 + uj) * 128 : (q * 2 + uj + 1) * 128]
                        for vh in range(2):
                            nc.tensor.matmul(
                                ps[:, uj * V + vh * 512 : uj * V + vh * 512 + 512],
                                lhs,
                                AC[:, h * V + vh * 512 : h * V + vh * 512 + 512],
                                start=True,
                                stop=True,
                            )
                    stage = ob.tile([128, 2 * V], FP16, tag="stage")
                    nc.vector.tensor_copy(stage[:, 0:V], ps[:, 0:V])
                    nc.scalar.copy(stage[:, V : 2 * V], ps[:, V : 2 * V])
                    nc.sync.dma_start(
                        out_d[k * 256 : (k + 1) * 256, :].rearrange(
                            "(p uj) v -> p (uj v)", uj=2
                        ),
                        stage[:],
                    )
                    if k == 0:
                        # deferred A half 1 -> AC[0:64, V:2V]
                        psA1 = po.tile([128, 2 * V], FP32, tag="psO")
                        for vh in range(2):
                            vs = slice(vh * 512, (vh + 1) * 512)
                            for mc in range(4):
                                nc.tensor.matmul(
                                    psA1[0:64, vs],
                                    tfT[mc][:, 64:128],
                                    Wj_sb[0][vh][:, mc * 512 : (mc + 1) * 512],
                                    start=(mc == 0),
                                    stop=(mc == 3),
                                )
                        nc.scalar.copy(AC[0:64, V : 2 * V], psA1[0:64, 0:V])

    nc.compile()
    return nc


def _swizzle(W):  # [512, F] -> SBUF image [128, 4*F]
    F = W.shape[1]
    return np.ascontiguousarray(
        W.reshape(4, 128, F).transpose(1, 0, 2).reshape(128, 4 * F)
    )


def kernel(f, g, We, be, Wd, bd, Wj, bj):
    if "nc" not in _cache:
        _cache["nc"] = _build_nc()
    nc = _cache["nc"]

    f16 = lambda x: np.asarray(x, dtype=np.float16)
    f, g = np.asarray(f), np.asarray(g)
    Wj16 = f16(Wj)
    shared = {
        "We_i": _swizzle(f16(We)),
        "Wd_i": _swizzle(f16(Wd)),
        "be_i": _swizzle(np.asarray(be, np.float32).reshape(512, 1)),
        "bd_i": _swizzle(np.asarray(bd, np.float32).reshape(512, 1)),
        "bj": np.ascontiguousarray(f16(bj)),
    }
    for r in range(2):
        for v in range(2):
            shared[f"Wj{r}{v}"] = _swizzle(
                Wj16[r * 512 : (r + 1) * 512, v * 512 : (v + 1) * 512]
            )
    in_maps = []
    for c in range(NCORES):
        b, th = c // 2, c % 2
        in_maps.append(
            {
                "fT_c": _swizzle(f16(f[b, th * TC : (th + 1) * TC, :]).T),
                "gT_c": _swizzle(f16(g[b]).T),
                **shared,
            }
        )
    res = run_bass_kernel_spmd(nc, in_maps, list(range(NCORES)))
    kernel._last_results = res

    out = np.empty((B, T, U, V), np.float32)
    for c in range(NCORES):
        b, th = c // 2, c % 2
        out[b, th * TC : (th + 1) * TC] = (
            res.results[c]["out"].reshape(TC, U, V).astype(np.float32)
        )
    return out


# revision 8
# speedup vs baseline: 1.1831x; 1.1831x over previous
"""RNN-T JointNetwork kernel for 8 Trainium2 NeuronCores.

Math: out[b,t,u,:] = tanh(concat(fe[b,t], gd[b,u])) @ Wj + bj
with fe = f@We+be, gd = g@Wd+bd.

tanh acts elementwise and the concat feeds one GEMM, so the joint GEMM
factorizes exactly:
    out[b,t,u,:] = A[b,t,:] + C[b,u,:]
    A = tanh(f@We+be) @ Wj[:Dm]          (per-(b,t) row)
    C = tanh(g@Wd+bd) @ Wj[Dm:] + bj     (per-(b,u) row)
leaving the kernel bound by the output write.  The rel-err budget (2e-2)
is huge, so the whole device pipeline runs in fp16 and the output is
written to DRAM as fp16 (half the HBM traffic of fp32); the host upcasts
while unsharding.

Sharding: 8 cores, core c owns (b = c//2, t-half = c%2) -> a [128,64,V]
output chunk per core (contiguous 16.8 MB fp16).

On-core plan:
  - all inputs arrive as exact fp16 SBUF images (host-side shard prep
    does the transpose/swizzle) so every input DMA is one flat block;
    issues are spread over both HWDGE queues (sync + scalar)
  - tfT[m,t] = tanh(We^T@f^T + be), tgT[m,u] likewise (PE + ACT)
  - AC[128, 2V] fp16: for each t-half h, rows 0:64 = A[64h:64h+64],
    rows 64:128 = C (+bj).  Evacuated from PSUM in v-half ladders so the
    main loop starts as early as possible; A-half1 is deferred past the
    first output tiles.
  - main loop: 32 output tiles of [128, 2048] covering 256 DRAM rows
    each; partition p holds DRAM rows 2p/2p+1 of the tile (u-pairs), so
    each output DMA descriptor is 4 KB contiguous (full write rate).
    Per tile: 4 matmuls, stationary selector picks A rows/replicates C
    (K=128, exact 0/1 weights built by gpsimd while inputs stream in).
    PSUM -> fp16 staging: DVE takes cols 0:1024, ACT cols 1024:2048,
    concurrently.  One 512 KB output DMA per tile.
"""

import sys

sys.path.insert(0, "/opt/trn_rl_repo")

import numpy as np

import concourse.bacc as bacc
import concourse.mybir as mybir
import concourse.tile as tile
from concourse.bass_utils import run_bass_kernel_spmd

B, T, U = 4, 256, 64
D = 512  # DE = DD = DM
V = 1024
TC = 128  # t rows per core
NCORES = 8
FP32 = mybir.dt.float32
FP16 = mybir.dt.float16
BF16 = mybir.dt.bfloat16
TANH = mybir.ActivationFunctionType.Tanh

_cache = {}


def _build_nc():
    nc = bacc.Bacc("TRN2", target_bir_lowering=False)

    # all fp16 inputs are pre-swizzled SBUF images [128, free]
    fT_d = nc.dram_tensor("fT_c", [128, 4 * TC], FP16, kind="ExternalInput")
    gT_d = nc.dram_tensor("gT_c", [128, 4 * U], FP16, kind="ExternalInput")
    We_d = nc.dram_tensor("We_i", [128, 4 * D], FP16, kind="ExternalInput")
    Wd_d = nc.dram_tensor("Wd_i", [128, 4 * D], FP16, kind="ExternalInput")
    # Wj images: [row-half][col-half], each [128, 4*512]
    Wj_d = [
        [
            nc.dram_tensor(f"Wj{r}{v}", [128, 4 * 512], FP16, kind="ExternalInput")
            for v in range(2)
        ]
        for r in range(2)
    ]
    be_d = nc.dram_tensor("be_i", [128, 4], FP32, kind="ExternalInput")
    bd_d = nc.dram_tensor("bd_i", [128, 4], FP32, kind="ExternalInput")
    bj_d = nc.dram_tensor("bj", [V], FP16, kind="ExternalInput")
    out_d = nc.dram_tensor("out", [TC * U, V], FP16, kind="ExternalOutput")

    with tile.TileContext(nc) as tc:
        with (
            tc.tile_pool(name="const", bufs=1) as cp,
            tc.tile_pool(name="wts", bufs=1) as wp,
        ):
            # Selector bank [128, 16*2*128]: slice (q, uj) is lhsT for
            # output tile k' (q = k'%16), column block uj.  Output
            # partition m holds DRAM rows 2m (uj=0) / 2m+1 (uj=1) of the
            # 256-row tile: t%64 = 4q + m//32, u = 2*(m%32) + uj.
            #   rows 0:64   (A): 1 iff r == 4q + m//32   (uj-independent)
            #   rows 64:128 (C): 1 iff r-64 == 2*(m%32) + uj
            selAC = cp.tile([128, 16 * 2 * 128], BF16, tag="selAC")
            nc.gpsimd.memset(selAC[:], 0.0)
            slA = selAC[0:64, :]
            nc.gpsimd.affine_select(
                out=slA.rearrange("p (q uj a c) -> p q uj a c", q=16, uj=2, a=4),
                in_=slA.rearrange("p (q uj a c) -> p q uj a c", q=16, uj=2, a=4),
                compare_op=mybir.AluOpType.not_equal,
                fill=1.0,
                base=0,
                pattern=[[-4, 16], [0, 2], [-1, 4], [0, 32]],
                channel_multiplier=1,
            )
            slC = selAC[64:128, :]
            nc.gpsimd.affine_select(
                out=slC.rearrange("p (q uj a c) -> p q uj a c", q=16, uj=2, a=4),
                in_=slC.rearrange("p (q uj a c) -> p q uj a c", q=16, uj=2, a=4),
                compare_op=mybir.AluOpType.not_equal,
                fill=1.0,
                base=0,
                pattern=[[0, 16], [-1, 2], [0, 4], [-2, 32]],
                channel_multiplier=1,
            )

            ones1 = cp.tile([1, 64], FP16, tag="ones1")
            nc.gpsimd.memset(ones1[:], 1.0)

            # ---- persistent operands ----
            fT_sb = wp.tile([128, 4 * TC], FP16, tag="fT")
            gT_sb = wp.tile([128, 4 * U], FP16, tag="gT")
            We_sb = wp.tile([128, 4 * D], FP16, tag="We")
            Wd_sb = wp.tile([128, 4 * D], FP16, tag="Wd")
            Wj_sb = [
                [
                    wp.tile([128, 4 * 512], FP16, tag=f"Wj{r}{v}", name=f"Wj{r}{v}")
                    for v in range(2)
                ]
                for r in range(2)
            ]
            be_sb = wp.tile([128, 4], FP32, tag="be")
            bd_sb = wp.tile([128, 4], FP32, tag="bd")
            bj_sb = wp.tile([1, V], FP16, tag="bj")
            tfT = [wp.tile([128, TC], FP16, tag=f"tfT{c}", name=f"tfT{c}") for c in range(4)]
            tgT = [wp.tile([128, U], FP16, tag=f"tgT{c}", name=f"tgT{c}") for c in range(4)]
            # AC[:, h*V:(h+1)*V]: rows 0:64 = A[64h:64h+64], rows 64:128 = C
            AC = wp.tile([128, 2 * V], BF16, tag="AC")

            # small/early inputs on the scalar HWDGE queue, big weights on
            # sync: the two queues issue in parallel.
            nc.scalar.dma_start(fT_sb[:], fT_d[:])
            nc.scalar.dma_start(be_sb[:], be_d[:])
            nc.scalar.dma_start(gT_sb[:], gT_d[:])
            nc.scalar.dma_start(bd_sb[:], bd_d[:])
            nc.scalar.dma_start(bj_sb[:], bj_d.rearrange("(p v) -> p v", p=1))
            nc.sync.dma_start(We_sb[:], We_d[:])
            nc.sync.dma_start(Wj_sb[0][0][:], Wj_d[0][0][:])
            nc.sync.dma_start(Wj_sb[0][1][:], Wj_d[0][1][:])
            nc.sync.dma_start(Wd_sb[:], Wd_d[:])
            nc.sync.dma_start(Wj_sb[1][0][:], Wj_d[1][0][:])
            nc.sync.dma_start(Wj_sb[1][1][:], Wj_d[1][1][:])

            # ---- prologue: tfT, tgT, then AC (A-half0 + C), laddered ----
            with tc.tile_pool(name="pp", bufs=4, space="PSUM") as pp:
                for mc in range(4):
                    ps = pp.tile([128, TC], FP32, tag="pps")
                    for dc in range(4):
                        nc.tensor.matmul(
                            ps[:],
                            We_sb[:, dc * D + mc * 128 : dc * D + (mc + 1) * 128],
                            fT_sb[:, dc * TC : (dc + 1) * TC],
                            start=(dc == 0),
                            stop=(dc == 3),
                        )
                    nc.scalar.activation(
                        tfT[mc][:], ps[:], TANH, bias=be_sb[:, mc : mc + 1]
                    )
                for mc in range(4):
                    ps = pp.tile([128, U], FP32, tag="pps")
                    for dc in range(4):
                        nc.tensor.matmul(
                            ps[:],
                            Wd_sb[:, dc * D + mc * 128 : dc * D + (mc + 1) * 128],
                            gT_sb[:, dc * U : (dc + 1) * U],
                            start=(dc == 0),
                            stop=(dc == 3),
                        )
                    nc.scalar.activation(
                        tgT[mc][:], ps[:], TANH, bias=bd_sb[:, mc : mc + 1]
                    )

                # A half 0 -> psum rows 0:64, evacuated per v-half (DVE)
                psA = pp.tile([64, V], FP32, tag="pps")
                for vh in range(2):
                    vs = slice(vh * 512, (vh + 1) * 512)
                    for mc in range(4):
                        nc.tensor.matmul(
                            psA[:, vs],
                            tfT[mc][:, 0:64],
                            Wj_sb[0][vh][:, mc * 512 : (mc + 1) * 512],
                            start=(mc == 0),
                            stop=(mc == 3),
                        )
                    nc.vector.tensor_copy(AC[0:64, vs], psA[:, vs])

                # C -> psum rows 64:128, + bj row; h0 slice per v-half on
                # ACT (laddered), h1 slice in one DVE copy (needed later)
                psC = pp.tile([128, V], FP32, tag="pps")
                for vh in range(2):
                    vs = slice(vh * 512, (vh + 1) * 512)
                    for mc in range(4):
                        nc.tensor.matmul(
                            psC[64:128, vs],
                            tgT[mc][:],
                            Wj_sb[1][vh][:, mc * 512 : (mc + 1) * 512],
                            start=(mc == 0),
                            stop=False,
                            tile_position=(0, 64),
                        )
                    nc.tensor.matmul(
                        psC[64:128, vs],
                        ones1[:],
                        bj_sb[:, vs],
                        start=False,
                        stop=True,
                        tile_position=(0, 64),
                    )
                    nc.scalar.copy(AC[64:128, vs], psC[64:128, vs])
                nc.vector.tensor_copy(AC[64:128, V : 2 * V], psC[64:128, :])

            # ---- main loop: 32 output tiles of [128, 2048] fp16 ----
            # tile k' covers DRAM rows [256k', 256k'+256); partition p
            # holds rows 2p/2p+1 as column halves -> 4 KB descriptors.
            with (
                tc.tile_pool(name="po", bufs=4, space="PSUM") as po,
                tc.tile_pool(name="ob", bufs=6) as ob,
            ):
                for k in range(32):
                    h, q = k // 16, k % 16
                    stage = ob.tile([128, 2 * V], FP16, tag="stage")
                    for uj in range(2):
                        ps = po.tile([128, V], FP32, tag="psO")
                        lhs = selAC[:, (q * 2 + uj) * 128 : (q * 2 + uj + 1) * 128]
                        for vh in range(2):
                            nc.tensor.matmul(
                                ps[:, vh * 512 : vh * 512 + 512],
                                lhs,
                                AC[:, h * V + vh * 512 : h * V + vh * 512 + 512],
                                start=True,
                                stop=True,
                            )
                        if uj == 0:
                            nc.vector.tensor_copy(stage[:, 0:V], ps[:])
                        else:
                            nc.scalar.copy(stage[:, V : 2 * V], ps[:])
                    nc.sync.dma_start(
                        out_d[k * 256 : (k + 1) * 256, :].rearrange(
                            "(p uj) v -> p (uj v)", uj=2
                        ),
                        stage[:],
                    )
                    if k == 0:
                        # deferred A half 1 -> AC[0:64, V:2V]
                        psA1 = po.tile([128, V], FP32, tag="psO")
                        for vh in range(2):
                            vs = slice(vh * 512, (vh + 1) * 512)
                            for mc in range(4):
                                nc.tensor.matmul(
                                    psA1[0:64, vs],
                                    tfT[mc][:, 64:128],
                                    Wj_sb[0][vh][:, mc * 512 : (mc + 1) * 512],
                                    start=(mc == 0),
                                    stop=(mc == 3),
                                )
                        nc.scalar.copy(AC[0:64, V : 2 * V], psA1[0:64, 0:V])

    nc.compile()
    return nc


def _swizzle(W):  # [512, F] -> SBUF image [128, 4*F]
    F = W.shape[1]
    return np.ascontiguousarray(
        W.reshape(4, 128, F).transpose(1, 0, 2).reshape(128, 4 * F)
    )


def kernel(f, g, We, be, Wd, bd, Wj, bj):
    if "nc" not in _cache:
        _cache["nc"] = _build_nc()
    nc = _cache["nc"]

    f16 = lambda x: np.asarray(x, dtype=np.float16)
    f, g = np.asarray(f), np.asarray(g)
    Wj16 = f16(Wj)
    shared = {
        "We_i": _swizzle(f16(We)),
        "Wd_i": _swizzle(f16(Wd)),
        "be_i": _swizzle(np.asarray(be, np.float32).reshape(512, 1)),
        "bd_i": _swizzle(np.asarray(bd, np.float32).reshape(512, 1)),
        "bj": np.ascontiguousarray(f16(bj)),
    }
    for r in range(2):
        for v in range(2):
            shared[f"Wj{r}{v}"] = _swizzle(
                Wj16[r * 512 : (r + 1) * 512, v * 512 : (v + 1) * 512]
            )
    in_maps = []
    for c in range(NCORES):
        b, th = c // 2, c % 2
        in_maps.append(
            {
                "fT_c": _swizzle(f16(f[b, th * TC : (th + 1) * TC, :]).T),
                "gT_c": _swizzle(f16(g[b]).T),
                **shared,
            }
        )
    res = run_bass_kernel_spmd(nc, in_maps, list(range(NCORES)))
    kernel._last_results = res

    out = np.empty((B, T, U, V), np.float32)
    for c in range(NCORES):
        b, th = c // 2, c % 2
        out[b, th * TC : (th + 1) * TC] = (
            res.results[c]["out"].reshape(TC, U, V).astype(np.float32)
        )
    return out
